# revision 1
# baseline (speedup 1.0000x reference)
"""Trainium2 Bass kernel: batched single-head attention.

Reference computation (per batch b):
    q = x @ Wq + bq ; k = x @ Wk + bk ; v = x @ Wv + bv      # [S, H]
    out = softmax((q k^T) / sqrt(H)) @ v                     # [S, H]

Shapes: B=4, S=4096, D_IN=512, D_H=64, fp32.

Sharding: 8 cores = (batch, query-half). Core c handles batch c//2,
queries (c%2)*2048 .. +2048. Host-side prep rotates x[b] so each core's
queries are always rows 0:2048 of its shard (softmax over keys is
permutation-invariant), and pre-transposes to x^T [512, 4096] so the
on-device matmuls can contract over D_IN on the partition dim without
any on-device transpose of x.

On-device dataflow per core (all matmuls run as float32r; 1 cyc/row):
  KV^T[128,s]   = [Wk|Wv]^T x^T + [bk;bv]     (PE->psum, DVE bias-copy)
  Q^T [64,2048] = Wq^T x^T[:, :2048] + bq     (q-chunks 0-3 only)
  V_nat[128,kt,65] = PE-transpose of V^T rows; col 64 = ones (denominator)
  per key-tile kt (32 x 128 keys), in halves h of 1024 queries:
    S^T[128,1024] = K^T_kt^T Q^T                             (PE -> psum)
    P^T[128,1024] = exp(0.125 * S^T)                         (ACT, fused scale)
    out^T[65,2048] += V_ext_kt^T P^T                         (PE, psum accum)
  K/V projections for s-chunks 4-7 are interleaved into the first
  attention iterations (kt 0..15 only need chunks 0-3) so the x^T DMA
  overlaps the ACT-bound attention loop.
  out^T row 64 = softmax denominators; shipped as-is (yT [65, 2048]),
  host does y = (yT[:64] / yT[64]).T  (tiny, avoids on-device
  transpose+reciprocal tail).
"""

import numpy as np

B, S, D_IN, D_H = 4, 4096, 512, 64
QW = S // 2          # queries per core
N_CORES = 8
NKT = S // 128       # 32 key tiles
NQC = QW // 512      # 4 query chunks of 512
NSC = S // 512       # 8 s chunks of 512
NDT = D_IN // 128    # 4 contraction tiles
HW = QW // 2         # 1024-wide attention half-tiles


def build_nc(repeats=1, HEAD_ALL=False):
    """Build + compile the Bacc module for one core (SPMD across 8)."""
    import concourse.bass as bass
    import concourse.tile as tile
    from concourse import bacc, mybir

    f32 = mybir.dt.float32
    f32r = mybir.dt.float32r
    EXP = mybir.ActivationFunctionType.Exp

    nc = bacc.Bacc("TRN2", target_bir_lowering=False, debug=False,
                   num_devices=N_CORES)

    xT_d = nc.dram_tensor("xT", (D_IN, S), f32r, kind="ExternalInput").ap()
    w_d = nc.dram_tensor("w", (D_IN, 192), f32r, kind="ExternalInput").ap()
    cst_d = nc.dram_tensor("consts", (128, 132), f32r,
                           kind="ExternalInput").ap()
    yT_d = nc.dram_tensor("yT", (65, QW), f32, kind="ExternalOutput").ap()

    with tile.TileContext(nc) as tc:
        import contextlib
        with contextlib.ExitStack() as ctx:
            sb = ctx.enter_context(tc.tile_pool(name="sb", bufs=1))
            ptp = ctx.enter_context(tc.tile_pool(name="ptp", bufs=4))

            # ---- persistent buffers (DMAs issued below, interleaved
            # with the x^T chunk loads for head latency) ----
            w_sb = sb.tile([128, NDT, 192], f32r)      # [Wk|Wv|Wq] d-tiles
            cst_sb = sb.tile([128, 132], f32r)         # eye|ones|pad|bkv|bq
            xt = sb.tile([128, NDT, S], f32r)          # x^T tiles
            kvt = sb.tile([128, S], f32r)              # rows 0:64 K^T, 64:128 V^T
            qt_sb = sb.tile([128, QW], f32r)           # rows 0:64 Q^T
            vnat = sb.tile([128, NKT, 65], f32r)       # V natural + ones col
            yT_sb = sb.tile([128, QW], f32)
            warm_sb = sb.tile([128, 4], f32)

            id_sb = cst_sb[:, 0:128]
            bkv_sb = cst_sb[:, 130:131].bitcast(f32)
            bq_sb = cst_sb[:, 131:132].bitcast(f32)

            for _rep in range(repeats):
              with tc.tile_pool(name=f"pa{_rep}", bufs=1, space="PSUM") as pa:
                # DMA queue order = completion order: weights, the four
                # q-critical x^T chunks, consts (identity/biases), the
                # vnat ones column, then the remaining x^T chunks.
                nc.sync.dma_start(w_sb, w_d.rearrange("(t p) m -> p t m",
                                                      p=128))
                nc.sync.dma_start(cst_sb, cst_d)
                xT_r = xT_d.rearrange("(t p) s -> p t s", p=128)
                for c in range(NQC):
                    cs = slice(512 * c, 512 * (c + 1))
                    nc.sync.dma_start(xt[:, :, cs], xT_r[:, :, cs])
                for c in range(NQC, NSC):
                    cs = slice(512 * c, 512 * (c + 1))
                    nc.sync.dma_start(xt[:, :, cs], xT_r[:, :, cs])

                # warm-ups: pre-touch operands one semaphore at a time (walrus
                # allows at most ONE sync wait per engine instruction)
                nc.scalar.activation(warm_sb[0:1, 2:3], warm_sb[0:1, 3:4], EXP,
                                     scale=1.0)
                nc.vector.tensor_copy(warm_sb[:, 0:1], bkv_sb)
                # vnat denominator column: broadcast the resident ones column
                # (DVE, ~0.1us) instead of a scattered 0-stride DMA (1.8us
                # that also delayed the chunk 4-7 loads behind it)
                ones_col = bass.AP(tensor=cst_sb.tensor, offset=cst_sb.offset
                                   + 128, ap=[[132, 128], [0, NKT], [1, 1]])
                nc.vector.tensor_copy(vnat[:, :, 64:65], ones_col)
                warm = pa.tile([128, 132], f32, tag="st", bufs=2)
                nc.tensor.matmul(warm[:, 0:2], lhsT=w_sb[:, 0, 0:128],
                                 rhs=w_sb[:, 0, 0:2], start=True, stop=True)
                nc.tensor.transpose(warm[0:1, 4:132].bitcast(f32r),
                                    in_=id_sb[:, 0:1], identity=id_sb)
                # HAM warm-up: sustained junk matmuls on already-loaded
                # weights keep PE busy through the x^T DMA wait so the first
                # S^T matmuls run at 2.4 GHz (cold-PE costs ~3.5 us otherwise)
                for _ in range(12):
                    nc.tensor.matmul(warm[:, 0:128], lhsT=w_sb[:, 0, 0:128],
                                     rhs=w_sb[:, 0, 0:128], start=True,
                                     stop=True)

                def proj_kv(c, tag="st"):
                    cs = slice(512 * c, 512 * (c + 1))
                    pkv = pa.tile([128, HW], f32, tag=tag,
                                  bufs=(2 if tag == "st" else 1), name="pkv")
                    for dt in range(NDT):
                        nc.tensor.matmul(
                            pkv[:, 0:512],
                            lhsT=w_sb[:, dt, 0:128], rhs=xt[:, dt, cs],
                            start=(dt == 0), stop=(dt == NDT - 1))
                    nc.vector.tensor_scalar_add(kvt[:, cs], pkv[:, 0:512],
                                                bkv_sb)

                def proj_q(c, tag="st"):
                    cs = slice(512 * c, 512 * (c + 1))
                    pq = pa.tile([128, HW], f32, tag=tag,
                                 bufs=(2 if tag == "st" else 1), name="pq")
                    for dt in range(NDT):
                        nc.tensor.matmul(
                            pq[0:D_H, 0:512],
                            lhsT=w_sb[:, dt, 128:192], rhs=xt[:, dt, cs],
                            start=(dt == 0), stop=(dt == NDT - 1))
                    nc.vector.tensor_scalar_add(
                        qt_sb[0:D_H, cs], pq[0:D_H, 0:512], bq_sb[0:D_H, :])

                def v_nat(c, tag="st"):
                    pvt = pa.tile([128, HW], f32r, tag=tag,
                                  bufs=(2 if tag == "st" else 1), name="pvt")
                    for j in range(4):
                        kt = 4 * c + j
                        nc.tensor.transpose(
                            pvt[:, D_H * j:D_H * (j + 1)],
                            in_=kvt[64:128, 128 * kt:128 * (kt + 1)],
                            identity=id_sb[64:128, 64:128])
                    nc.vector.tensor_copy(
                        vnat[:, 4 * c:4 * (c + 1), 0:D_H],
                        pvt[:, 0:4 * D_H].rearrange("p (t h) -> p t h", h=D_H))
                    # junk matmul: advances the PE engine clock past the vnat
                    # copy's DVE tick (walrus 1-wait limit on later AV MMs)
                    nc.tensor.matmul(
                        pvt[0:65, 0:2].bitcast(f32), lhsT=vnat[:, 4 * c, :],
                        rhs=vnat[:, 4 * c, 0:2], start=True, stop=True)

                def chunk_work(c):
                    # kv projection + V transpose of one s-chunk in a single
                    # outB slot hold (halves the serialized-slot chain)
                    cs = slice(512 * c, 512 * (c + 1))
                    t = pa.tile([128, HW], f32, tag="outB", bufs=1, name="cw")
                    for dt in range(NDT):
                        nc.tensor.matmul(
                            t[:, 0:512],
                            lhsT=w_sb[:, dt, 0:128], rhs=xt[:, dt, cs],
                            start=(dt == 0), stop=(dt == NDT - 1))
                    nc.vector.tensor_scalar_add(kvt[:, cs], t[:, 0:512],
                                                bkv_sb)
                    for j in range(4):
                        kt = 4 * c + j
                        nc.tensor.transpose(
                            t[:, 512 + D_H * j:512 + D_H * (j + 1)].bitcast(f32r),
                            in_=kvt[64:128, 128 * kt:128 * (kt + 1)],
                            identity=id_sb[64:128, 64:128])
                    nc.vector.tensor_copy(
                        vnat[:, 4 * c:4 * (c + 1), 0:D_H],
                        t[:, 512:512 + 4 * D_H].bitcast(f32r)
                        .rearrange("p (t h) -> p t h", h=D_H))
                    # junk matmul: advances the PE engine clock past the vnat
                    # copy's DVE tick (walrus 1-wait limit on later AV MMs)
                    nc.tensor.matmul(
                        t[0:65, 768:770], lhsT=vnat[:, 4 * c, :],
                        rhs=vnat[:, 4 * c, 0:2], start=True, stop=True)

                # head variant (A): everything before the attention loop
                if HEAD_ALL:
                    for c in range(NSC):
                        proj_kv(c)
                        if c < NQC:
                            proj_q(c)
                        v_nat(c)
                else:
                    # head: the h=0 attention sub-pipeline only needs q-chunks
                    # 0-1 and kvt/V of chunks 0-1 -- emitted in chunk-arrival
                    # order so the PE ops hide inside the x^T DMA wait.
                    # q-chunks 2-3 (only needed by h=1 jobs) become extras.
                    proj_kv(0)
                    v_nat(0)
                    proj_q(0)
                    proj_q(1)
                    proj_kv(1)

                poutA = pa.tile([65, HW], f32, tag="outA")

                def st_tile(kt, h):
                    pst = pa.tile([128, HW], f32, tag="st", bufs=2,
                                  name=f"pst_{kt}_{h}")
                    for c in range(2):
                        cs = slice(512 * c, 512 * (c + 1))
                        qs = slice(HW * h + 512 * c, HW * h + 512 * (c + 1))
                        nc.tensor.matmul(
                            pst[:, cs],
                            lhsT=kvt[0:64, 128 * kt:128 * (kt + 1)],
                            rhs=qt_sb[0:64, qs],
                            start=True, stop=True)
                    return pst

                # flat job order: (kt, h) with h=1 lagging 6 kt behind h=0,
                # so the ACT engine starts on h=0 tiles ~8 us earlier while
                # x^T chunks 2-3 (needed by q-half 1) are still streaming
                LAG = 24
                jobs = [(k, 0) for k in range(LAG)]
                for i in range(NKT - LAG):
                    jobs += [(i, 1), (i + LAG, 0)]
                jobs += [(k, 1) for k in range(NKT - LAG, NKT)]
                assert len(jobs) == 2 * NKT

                # work interleaved into early iterations: kv proj + V
                # transpose for s-chunks 1-7; chunk c's K^T is needed by
                # S^T(4c) emitted in iteration 4c-1, its V by AV(4c); the
                # extras run at iteration 2c-1 / 2c -- always well ahead
                # deadline-paced: chunk c's K^T is needed by S^T(4c, h0)
                # emitted at job 4c-2, so late chunks run in the PE-slack era
                extra_at = {}
                if not HEAD_ALL:
                    extra_at = {
                        1: lambda: v_nat(1, tag="outB"),
                        2: lambda: proj_q(2, tag="outB"),
                        3: lambda: proj_q(3, tag="outB"),
                        4: lambda: chunk_work(2),
                        8: lambda: chunk_work(3),
                        12: lambda: chunk_work(4),
                        16: lambda: chunk_work(5),
                        19: lambda: chunk_work(6),
                        22: lambda: chunk_work(7),
                    }

                pouts = [poutA, None]
                psts = {0: st_tile(*jobs[0]), 1: st_tile(*jobs[1])}
                for j in range(2 * NKT):
                    kt, h = jobs[j]
                    if h == 1 and pouts[1] is None:
                        pouts[1] = pa.tile([65, HW], f32, tag="outB",
                                           name="poutB")
                    pt = ptp.tile([128, HW], f32r, tag="pt", name="ptile")
                    nc.scalar.activation(pt, psts.pop(j), EXP, scale=0.125)
                    if j + 2 < 2 * NKT:
                        psts[j + 2] = st_tile(*jobs[j + 2])
                    if j in extra_at:
                        extra_at.pop(j)()
                    for cc in range(2):
                        cs = slice(512 * cc, 512 * (cc + 1))
                        nc.tensor.matmul(
                            pouts[h][:, cs],
                            lhsT=vnat[:, kt, :],
                            rhs=pt[:, cs],
                            start=(kt == 0), stop=(kt == NKT - 1),
                            skip_group_check=True)
                assert not extra_at

                # ship out^T + denominator row; host normalizes.
                # split in halves so the DMA overlaps the second copy
                for hh in range(2):
                    hs = slice(HW * hh, HW * (hh + 1))
                    nc.vector.tensor_copy(yT_sb[0:65, hs], pouts[hh])
                    nc.sync.dma_start(yT_d[:, hs], yT_sb[0:65, hs])

    nc.compile()
    return nc


def _prep_core_inputs(c, x, Wq, bq, Wk, bk, Wv, bv):
    b, qh = c // 2, c % 2
    xb = x[b]
    if qh:
        xb = np.concatenate([xb[QW:], xb[:QW]], axis=0)
    consts = np.zeros((128, 132), np.float32)
    consts[:, 0:128] = np.eye(128, dtype=np.float32)
    consts[:, 128] = 1.0                      # vnat denominator column
    consts[:, 130] = np.concatenate([bk, bv])  # [bk;bv] per-partition bias
    consts[0:D_H, 131] = bq
    return {
        "xT": np.ascontiguousarray(xb.T),
        "w": np.ascontiguousarray(np.concatenate([Wk, Wv, Wq], axis=1)),
        "consts": consts,
    }


def gather_output(per_core_yT):
    """per_core_yT: list of 8 arrays [65, QW] -> full y [B, S, D_H]."""
    y = np.empty((B, S, D_H), np.float32)
    for c in range(N_CORES):
        b, qh = c // 2, c % 2
        yT = np.asarray(per_core_yT[c])
        y[b, qh * QW:(qh + 1) * QW] = (yT[0:D_H] / yT[D_H:D_H + 1]).T
    return y


def run(x, Wq, bq, Wk, bk, Wv, bv, trace=False):
    """Returns (y [B,S,H], BassKernelResults)."""
    from concourse import bass_utils

    x = np.asarray(x, np.float32)
    in_maps = [
        _prep_core_inputs(c, x, np.asarray(Wq, np.float32),
                          np.asarray(bq, np.float32), np.asarray(Wk, np.float32),
                          np.asarray(bk, np.float32), np.asarray(Wv, np.float32),
                          np.asarray(bv, np.float32))
        for c in range(N_CORES)
    ]
    nc = build_nc()
    res = bass_utils.run_bass_kernel_spmd(
        nc, in_maps, core_ids=list(range(N_CORES)), trace=trace)
    y = gather_output([res.results[c]["yT"] for c in range(N_CORES)])
    return y, res


def kernel(x, Wq, bq, Wk, bk, Wv, bv):
    y, _ = run(x, Wq, bq, Wk, bk, Wv, bv, trace=False)
    return y



# revision 19
# speedup vs baseline: 53.8886x; 53.8886x over previous
"""Trainium2 Bass kernel: batched single-head attention.

Reference computation (per batch b):
    q = x @ Wq + bq ; k = x @ Wk + bk ; v = x @ Wv + bv      # [S, H]
    out = softmax((q k^T) / sqrt(H)) @ v                     # [S, H]

Shapes: B=4, S=4096, D_IN=512, D_H=64, fp32 reference; on-device the
x/W/K/Q/V/P datapath runs in bf16 (halves the head-critical DMA bytes;
matmul throughput is identical at 1 cyc/row; rel-err ~7e-3 « 2e-2
budget), with all PSUM accumulation in fp32.

Sharding: 8 cores = (batch, query-half). Core c handles batch c//2,
queries (c%2)*2048 .. +2048. Host-side prep rotates x[b] so each core's
queries are always rows 0:2048 of its shard (softmax over keys is
permutation-invariant), and pre-transposes to x^T [512, 4096] bf16.

On-device dataflow per core:
  KV^T[128,s]   = [Wk|Wv]^T x^T + [bk;bv]     (PE->psum, DVE bias-copy)
  Q^T [64,2048] = Wq^T x^T[:, :2048] + bq     (q-chunks 0-3 only)
  V_nat[128,kt,65] = PE-transpose of V^T rows; col 64 = ones (denominator)
  per key-tile kt (32 x 128 keys), in halves h of 1024 queries:
    S^T[128,1024] = K^T_kt^T Q^T                             (PE -> psum)
    P^T[128,1024] = exp(0.125 * S^T)  -> bf16                (ACT)
    out^T[65,2048] += V_ext_kt^T P^T                         (PE, psum f32)

Schedule: input DMAs stream in need-order on the SP queue (w, consts,
x^T chunks 0-7); PE junk-warms on w until chunk 0 lands (keeps the
p-state ramp hot), then the minimal chain proj_kv(0) -> proj_q(0) ->
proj_q(1) -> S^T(kt0,h0) starts the ACT exp stream ~8us in.  All
remaining K/V/Q prep is interleaved as deadline-paced extras inside the
h=0 jobs (kt 0..31), h=1 jobs follow.  out^T accumulates in PSUM;
q-half 0 ships (DVE copy + DMA) at job 31, q-half 1 in two pipelined
512-col pieces at the end.  out^T row 64 = softmax denominators; host
does y = (yT[:64] / yT[64]).T.
"""

import numpy as np

B, S, D_IN, D_H = 4, 4096, 512, 64
QW = S // 2          # queries per core
N_CORES = 8
NKT = S // 128       # 32 key tiles
NQC = QW // 512      # 4 query chunks of 512
NSC = S // 512       # 8 s chunks of 512
NDT = D_IN // 128    # 4 contraction tiles
HW = QW // 2         # 1024-wide attention half-tiles
N_JUNK = 11           # PE p-state warm matmuls (fill w-arrival .. chunk0)
CW = 212             # consts: eye32|ones|pk|bkv|bq|eye16(64)|ones16(16)


def build_nc(repeats=1):
    """Build + compile the Bacc module for one core (SPMD across 8)."""
    import concourse.bass as bass
    import concourse.tile as tile
    from concourse import bacc, mybir

    f32 = mybir.dt.float32
    f32r = mybir.dt.float32r
    bf16 = mybir.dt.bfloat16
    EXP = mybir.ActivationFunctionType.Exp

    nc = bacc.Bacc("TRN2", target_bir_lowering=False, debug=False,
                   num_devices=N_CORES)

    xT_d = nc.dram_tensor("xT", (D_IN, S), bf16, kind="ExternalInput").ap()
    w_d = nc.dram_tensor("w", (D_IN, 192), bf16, kind="ExternalInput").ap()
    cst_d = nc.dram_tensor("consts", (128, CW), f32r,
                           kind="ExternalInput").ap()
    yT_d = nc.dram_tensor("yT", (65, QW), f32, kind="ExternalOutput").ap()

    with tile.TileContext(nc) as tc:
        import contextlib
        with contextlib.ExitStack() as ctx:
            sb = ctx.enter_context(tc.tile_pool(name="sb", bufs=1))
            ptp = ctx.enter_context(tc.tile_pool(name="ptp", bufs=4))

            # ---- persistent buffers ----
            w_sb = sb.tile([128, NDT, 192], bf16)      # [Wk|Wv|Wq] d-tiles
            cst_sb = sb.tile([128, CW], f32r)          # eye|ones|bkv|bq|eye16
            xt = sb.tile([128, NDT, S], bf16)          # x^T tiles
            kvt = sb.tile([128, S], bf16)              # rows 0:64 K^T, 64:128 V^T
            vT32 = sb.tile([64, S], f32r)              # V^T rows, f32 (for PE transpose)
            qt_sb = sb.tile([128, QW], bf16)           # rows 0:64 Q^T
            vnat = sb.tile([128, NKT, 65], bf16)       # V natural + ones col
            yT_sb = sb.tile([128, QW], f32)
            warm_sb = sb.tile([128, 4], f32)
            junk_sb = sb.tile([128, 512], bf16)   # never written: junk warms

            bkv_sb = cst_sb[:, 130:131].bitcast(f32)
            bq_sb = cst_sb[:, 131:132].bitcast(f32)
            id_sb = cst_sb[:, 0:128]                   # f32r eye (transposes)
            id16 = cst_sb[:, 132:196].bitcast(bf16)    # [128, 128] bf16 eye
            w_fl = w_sb.rearrange("p t m -> p (t m)")

            for _rep in range(repeats):
              with tc.tile_pool(name=f"pa{_rep}", bufs=1, space="PSUM") as pa:
                # one SP-queue stream, arrival order = need order:
                # chunk 0 first (longest transfer on the critical path),
                # then w + consts, then chunks 1..7
                xT_r = xT_d.rearrange("(t p) s -> p t s", p=128)
                nc.sync.dma_start(xt[:, :, 0:512], xT_r[:, :, 0:512])
                nc.sync.dma_start(w_sb, w_d.rearrange("(t p) m -> p t m",
                                                      p=128))
                nc.sync.dma_start(cst_sb, cst_d)
                for c in range(1, NSC):
                    cs = slice(512 * c, 512 * (c + 1))
                    nc.sync.dma_start(xt[:, :, cs], xT_r[:, :, cs])

                # seed the junk/warm tiles via the idle Pool engine (tile
                # framework wants writers; Pool has no startup dependency,
                # so the PE junk warms can begin at t~0)
                nc.gpsimd.memset(junk_sb, 1.0)
                nc.gpsimd.memset(warm_sb, 1.0)
                # warm-ups: pre-touch operands one semaphore at a time (walrus
                # allows at most ONE sync wait per engine instruction)
                nc.scalar.activation(warm_sb[0:1, 2:3], warm_sb[0:1, 3:4], EXP,
                                     scale=1.0)
                nc.vector.tensor_copy(warm_sb[:, 0:1], bkv_sb)
                nc.vector.memset(vnat[:, :, 64:65], 1.0)
                warm = pa.tile([128, HW], f32, tag="st", bufs=2)
                # HAM warm-up: junk matmuls on an UNWRITTEN sbuf tile (no DMA
                # wait -> PE busy from t~0) keep the p-state ramp running
                # through the w/x^T chunk-0 DMA waits so the head projections
                # run as fast as possible
                for _ in range(N_JUNK):
                    nc.tensor.matmul(warm[:, 0:512], lhsT=junk_sb[:, 0:128],
                                     rhs=junk_sb, start=True, stop=True)

                def proj_kv(c, tag="st"):
                    cs = slice(512 * c, 512 * (c + 1))
                    pkv = pa.tile([128, HW], f32, tag=tag,
                                  bufs=(2 if tag == "st" else 1), name="pkv")
                    for dt in range(NDT):
                        nc.tensor.matmul(
                            pkv[:, 0:512],
                            lhsT=w_sb[:, dt, 0:128], rhs=xt[:, dt, cs],
                            start=(dt == 0), stop=(dt == NDT - 1))
                    nc.vector.tensor_scalar_add(kvt[:, cs], pkv[:, 0:512],
                                                bkv_sb)
                    nc.vector.tensor_scalar_add(
                        vT32[:, cs], pkv[64:128, 0:512],
                        bkv_sb[64:128, :])

                def proj_q(c, tag="st"):
                    cs = slice(512 * c, 512 * (c + 1))
                    pq = pa.tile([128, HW], f32, tag=tag,
                                 bufs=(2 if tag == "st" else 1), name="pq")
                    for dt in range(NDT):
                        nc.tensor.matmul(
                            pq[0:D_H, 0:512],
                            lhsT=w_sb[:, dt, 128:192], rhs=xt[:, dt, cs],
                            start=(dt == 0), stop=(dt == NDT - 1))
                    nc.vector.tensor_scalar_add(
                        qt_sb[0:D_H, cs], pq[0:D_H, 0:512], bq_sb[0:D_H, :])

                def v_nat(c):
                    pvt = pa.tile([128, HW], f32r, tag="outB", bufs=1,
                                  name="pvt")
                    for j in range(4):
                        kt = 4 * c + j
                        nc.tensor.transpose(
                            pvt[:, D_H * j:D_H * (j + 1)],
                            in_=vT32[:, 128 * kt:128 * (kt + 1)],
                            identity=id_sb[0:64, 0:64])
                    nc.vector.tensor_copy(
                        vnat[:, 4 * c:4 * (c + 1), 0:D_H],
                        pvt[:, 0:4 * D_H].rearrange("p (t h) -> p t h", h=D_H))
                    # junk matmul: advances the PE engine clock past the vnat
                    # copy's DVE tick (walrus 1-wait limit on later AV MMs)
                    nc.tensor.matmul(
                        pvt[0:65, 512:514].bitcast(f32), lhsT=vnat[:, 4 * c, :],
                        rhs=vnat[:, 4 * c, 0:2], start=True, stop=True)

                def chunk_work(c):
                    # kv projection + V transpose of one s-chunk in a single
                    # outB slot hold
                    cs = slice(512 * c, 512 * (c + 1))
                    t = pa.tile([128, HW], f32, tag="outB", bufs=1, name="cw")
                    for dt in range(NDT):
                        nc.tensor.matmul(
                            t[:, 0:512],
                            lhsT=w_sb[:, dt, 0:128], rhs=xt[:, dt, cs],
                            start=(dt == 0), stop=(dt == NDT - 1))
                    nc.vector.tensor_scalar_add(kvt[:, cs], t[:, 0:512],
                                                bkv_sb)
                    nc.vector.tensor_scalar_add(
                        vT32[:, cs], t[64:128, 0:512],
                        bkv_sb[64:128, :])
                    for j in range(4):
                        kt = 4 * c + j
                        nc.tensor.transpose(
                            t[:, 512 + D_H * j:512 + D_H * (j + 1)]
                            .bitcast(f32r),
                            in_=vT32[:, 128 * kt:128 * (kt + 1)],
                            identity=id_sb[0:64, 0:64])
                    nc.vector.tensor_copy(
                        vnat[:, 4 * c:4 * (c + 1), 0:D_H],
                        t[:, 512:512 + 4 * D_H].bitcast(f32r)
                        .rearrange("p (t h) -> p t h", h=D_H))
                    nc.tensor.matmul(
                        t[0:65, 768:770], lhsT=vnat[:, 4 * c, :],
                        rhs=vnat[:, 4 * c, 0:2], start=True, stop=True)

                # minimal head: only what S^T(kt0, h0) needs, so the ACT
                # exp stream starts as early as the x^T q-column DMAs allow
                proj_kv(0)
                proj_q(0)
                proj_q(1)

                poutA = pa.tile([65, HW], f32, tag="out", name="poutA")

                def st_tile(kt, h, tag="st"):
                    pst = pa.tile([128, HW], f32, tag=tag,
                                  bufs=(2 if tag == "st" else 1),
                                  name=f"pst_{kt}_{h}")
                    for c in range(2):
                        cs = slice(512 * c, 512 * (c + 1))
                        qs = slice(HW * h + 512 * c, HW * h + 512 * (c + 1))
                        nc.tensor.matmul(
                            pst[:, cs],
                            lhsT=kvt[0:64, 128 * kt:128 * (kt + 1)],
                            rhs=qt_sb[0:64, qs],
                            start=True, stop=True)
                    return pst

                # flat job order: all h=0 (kt 0..31), then all h=1
                jobs = [(k, 0) for k in range(NKT)] + [(k, 1) for k in
                                                       range(NKT)]
                NJ = 2 * NKT
                last_h0 = NKT - 1  # job 31

                # deadline-paced extras inside the h=0 jobs.  chunk c's K^T
                # is needed when S^T(4c, h0) is EMITTED (job 4c-2), its V by
                # AV(4c) (job 4c); q chunks 2-3 by the first h=1 S^T emission
                # (job 30); all outB users release before poutB allocates
                # (job 32).
                extra_at = {
                    0: lambda: v_nat(0),
                    1: lambda: proj_kv(1, tag="outB"),
                    2: lambda: v_nat(1),
                    5: lambda: chunk_work(2),
                    9: lambda: chunk_work(3),
                    13: lambda: chunk_work(4),
                    17: lambda: chunk_work(5),
                    21: lambda: chunk_work(6),
                    23: lambda: proj_q(2, tag="outB"),
                    25: lambda: chunk_work(7),
                    27: lambda: proj_q(3, tag="outB"),
                }

                def ship(h, pieces, engines=("v",)):
                    # pouts[h] -> SBUF -> DRAM, split for copy/DMA overlap;
                    # engines rotates the copy over DVE ("v") / ACT ("s")
                    w_ = HW // pieces
                    for p in range(pieces):
                        ps = slice(HW * h + w_ * p, HW * h + w_ * (p + 1))
                        ls = slice(w_ * p, w_ * (p + 1))
                        if engines[p % len(engines)] == "v":
                            nc.vector.tensor_copy(yT_sb[0:65, ps],
                                                  pouts[h][:, ls])
                        else:
                            nc.scalar.copy(yT_sb[0:65, ps], pouts[h][:, ls])
                        nc.sync.dma_start(yT_d[:, ps], yT_sb[0:65, ps])

                pouts = [poutA, None]
                psts = {0: st_tile(*jobs[0]), 1: st_tile(*jobs[1])}
                # warm the matmul-weight-load and transpose paths off the
                # critical chain (pre-touch w/id16 one semaphore at a time)
                nc.tensor.matmul(warm[:, 0:2], lhsT=w_sb[:, 0, 0:128],
                                 rhs=w_sb[:, 0, 0:2], start=True, stop=True)
                nc.tensor.transpose(warm[0:1, 4:132].bitcast(f32r),
                                    in_=id_sb[:, 0:1], identity=id_sb)
                for j in range(NJ):
                    kt, h = jobs[j]
                    if h == 1 and pouts[1] is None:
                        pouts[1] = pa.tile([65, HW], f32, tag="out",
                                           name="poutB")
                    pt = ptp.tile([128, HW], bf16, tag="pt", name="ptile")
                    nc.scalar.activation(pt, psts.pop(j), EXP, scale=0.125)
                    if j == 0:
                        psts[2] = st_tile(*jobs[2])
                    if j in extra_at:
                        extra_at.pop(j)()
                    for cc in range(2):
                        cs = slice(512 * cc, 512 * (cc + 1))
                        nc.tensor.matmul(
                            pouts[h][:, cs],
                            lhsT=vnat[:, kt, :],
                            rhs=pt[:, cs],
                            start=(kt == 0), stop=(kt == NKT - 1),
                            skip_group_check=True)
                    if j == last_h0:
                        ship(0, pieces=1)
                    if j + 3 < NJ:
                        m = j + 3
                        tag = "outB" if (m >= 33 and m % 3 == 0) else "st"
                        psts[m] = st_tile(*jobs[m], tag=tag)
                assert not extra_at
                ship(1, pieces=2, engines=("s", "v"))

    nc.compile()
    return nc


def _prep_core_inputs(c, x, Wq, bq, Wk, bk, Wv, bv):
    import ml_dtypes
    bf16 = ml_dtypes.bfloat16
    b, qh = c // 2, c % 2
    xb = x[b]
    if qh:
        xb = np.concatenate([xb[QW:], xb[:QW]], axis=0)
    consts = np.zeros((128, CW), np.float32)
    consts[:, 0:128] = np.eye(128, dtype=np.float32)
    consts[:, 128] = 1.0
    # col 129: two packed bf16 ones (vnat denominator column source)
    consts[:, 130] = np.concatenate([bk, bv])  # [bk;bv] per-partition bias
    consts[0:D_H, 131] = bq
    # cols 132:196: bf16 eye(128), packed 2-per-f32
    eye16 = np.eye(128, dtype=bf16).view(np.uint16).reshape(128, 64, 2)
    packed = (eye16[:, :, 0].astype(np.uint32)
              | (eye16[:, :, 1].astype(np.uint32) << 16))
    consts[:, 132:196] = packed.view(np.float32)
    # cols 196:212: 32 packed bf16 ones
    consts[:, 196:212] = np.array([0x3F803F80], np.uint32).view(np.float32)[0]
    return {
        "xT": np.ascontiguousarray(xb.T).astype(bf16),
        "w": np.ascontiguousarray(
            np.concatenate([Wk, Wv, Wq], axis=1)).astype(bf16),
        "consts": consts,
    }


def gather_output(per_core_yT):
    """per_core_yT: list of 8 arrays [65, QW] -> full y [B, S, D_H]."""
    y = np.empty((B, S, D_H), np.float32)
    for c in range(N_CORES):
        b, qh = c // 2, c % 2
        yT = np.asarray(per_core_yT[c])
        y[b, qh * QW:(qh + 1) * QW] = (yT[0:D_H] / yT[D_H:D_H + 1]).T
    return y


def run(x, Wq, bq, Wk, bk, Wv, bv, trace=False):
    """Returns (y [B,S,H], BassKernelResults)."""
    from concourse import bass_utils

    x = np.asarray(x, np.float32)
    in_maps = [
        _prep_core_inputs(c, x, np.asarray(Wq, np.float32),
                          np.asarray(bq, np.float32), np.asarray(Wk, np.float32),
                          np.asarray(bk, np.float32), np.asarray(Wv, np.float32),
                          np.asarray(bv, np.float32))
        for c in range(N_CORES)
    ]
    nc = build_nc()
    res = bass_utils.run_bass_kernel_spmd(
        nc, in_maps, core_ids=list(range(N_CORES)), trace=trace)
    y = gather_output([res.results[c]["yT"] for c in range(N_CORES)])
    return y, res


def kernel(x, Wq, bq, Wk, bk, Wv, bv):
    y, _ = run(x, Wq, bq, Wk, bk, Wv, bv, trace=False)
    return y


# revision 20
# speedup vs baseline: 55.9196x; 1.0377x over previous
"""Trainium2 Bass kernel: batched single-head attention.

Reference computation (per batch b):
    q = x @ Wq + bq ; k = x @ Wk + bk ; v = x @ Wv + bv      # [S, H]
    out = softmax((q k^T) / sqrt(H)) @ v                     # [S, H]

Shapes: B=4, S=4096, D_IN=512, D_H=64, fp32 reference; on-device the
x/W/K/Q/V/P datapath runs in bf16 (halves the head-critical DMA bytes;
matmul throughput is identical at 1 cyc/row; rel-err ~7e-3 « 2e-2
budget), with all PSUM accumulation in fp32.

Sharding: 8 cores = (batch, query-half). Core c handles batch c//2,
queries (c%2)*2048 .. +2048. Host-side prep rotates x[b] so each core's
queries are always rows 0:2048 of its shard (softmax over keys is
permutation-invariant), and pre-transposes to x^T [512, 4096] bf16.

On-device dataflow per core:
  KV^T[128,s]   = [Wk|Wv]^T x^T + [bk;bv]     (PE->psum, DVE bias-copy)
  Q^T [64,2048] = Wq^T x^T[:, :2048] + bq     (q-chunks 0-3 only)
  V_nat[128,kt,65] = PE-transpose of V^T rows; col 64 = ones (denominator)
  per key-tile kt (32 x 128 keys), in halves h of 1024 queries:
    S^T[128,1024] = K^T_kt^T Q^T                             (PE -> psum)
    P^T[128,1024] = exp(0.125 * S^T)  -> bf16                (ACT)
    out^T[65,2048] += V_ext_kt^T P^T                         (PE, psum f32)

Schedule: input DMAs stream in need-order on the SP queue (w, consts,
x^T chunks 0-7); PE junk-warms on w until chunk 0 lands (keeps the
p-state ramp hot), then the minimal chain proj_kv(0) -> proj_q(0) ->
proj_q(1) -> S^T(kt0,h0) starts the ACT exp stream ~8us in.  All
remaining K/V/Q prep is interleaved as deadline-paced extras inside the
h=0 jobs (kt 0..31), h=1 jobs follow.  out^T accumulates in PSUM;
q-half 0 ships (DVE copy + DMA) at job 31, q-half 1 in two pipelined
512-col pieces at the end.  out^T row 64 = softmax denominators; host
does y = (yT[:64] / yT[64]).T.
"""

import numpy as np


def _register_exp_op():
    """Custom DVE op: (c0*x + c1)^32 (affine + 5 squarings).  Two chained
    passes give (1 + z/1024)^1024 ~ exp(z) at <=1.2e-3 rel err for |z|<3.
    Registered into concourse.dve_ops at build time (new opcode row)."""
    from concourse import dve_ops
    from concourse.dve_ops import DveOp
    from concourse.dve_spec import Spec, Src0, C0, C1, sq

    name = "EXP_POW32_ANT"
    if name in dve_ops._SUB_OPCODE_FOR_NAME:
        return next(o for o in dve_ops.OPS if o.name == name)

    def _ref(in0, in1, c0, c1, c2):
        u = in0.astype(np.float32) * np.float32(c0) + np.float32(c1)
        for _ in range(5):
            u = (u * u).astype(np.float32)
        return u

    op = DveOp(name, Spec(body=sq(sq(sq(sq(sq(Src0 * C0 + C1))))),
                          reference=_ref),
               subdim=False,
               uops_sha={"v3": "eafb894a1d5c531b",
                         "v4": "305ddd2af0946706"})
    row = dve_ops._CUSTOM_DVE_ROW_BASE + len(dve_ops.OPS)
    assert row < 0x20
    dve_ops.OPS.append(op)
    dve_ops.CUSTOM_DVE_SPECS[name] = op.spec
    dve_ops._SUB_OPCODE_FOR_NAME[name] = row
    return op


B, S, D_IN, D_H = 4, 4096, 512, 64
QW = S // 2          # queries per core
N_CORES = 8
NKT = S // 128       # 32 key tiles
NQC = QW // 512      # 4 query chunks of 512
NSC = S // 512       # 8 s chunks of 512
NDT = D_IN // 128    # 4 contraction tiles
HW = QW // 2         # 1024-wide attention half-tiles
N_JUNK = 11          # PE p-state warm matmuls (fill w-arrival .. chunk0)
# h=0 jobs whose exp runs on DVE (2-pass pow32) instead of ACT: placed just
# after each chunk_work extra, where PE front-loading starves the ACT stream
OFFLOAD_EXP = (6, 10, 14, 18, 22, 26)
CW = 212             # consts: eye32|ones|pk|bkv|bq|eye16(64)|ones16(16)


def build_nc(repeats=1):
    """Build + compile the Bacc module for one core (SPMD across 8)."""
    import concourse.bass as bass
    import concourse.tile as tile
    from concourse import bacc, mybir

    f32 = mybir.dt.float32
    f32r = mybir.dt.float32r
    bf16 = mybir.dt.bfloat16
    EXP = mybir.ActivationFunctionType.Exp

    EXPOP = _register_exp_op()
    EXPC0 = 0.125 / 1024.0

    nc = bacc.Bacc("TRN2", target_bir_lowering=False, debug=False,
                   num_devices=N_CORES)

    xT_d = nc.dram_tensor("xT", (D_IN, S), bf16, kind="ExternalInput").ap()
    w_d = nc.dram_tensor("w", (D_IN, 192), bf16, kind="ExternalInput").ap()
    cst_d = nc.dram_tensor("consts", (128, CW), f32r,
                           kind="ExternalInput").ap()
    yT_d = nc.dram_tensor("yT", (65, QW), f32, kind="ExternalOutput").ap()

    with tile.TileContext(nc) as tc:
        import contextlib
        with contextlib.ExitStack() as ctx:
            sb = ctx.enter_context(tc.tile_pool(name="sb", bufs=1))
            ptp = ctx.enter_context(tc.tile_pool(name="ptp", bufs=4))

            # ---- persistent buffers ----
            w_sb = sb.tile([128, NDT, 192], bf16)      # [Wk|Wv|Wq] d-tiles
            cst_sb = sb.tile([128, CW], f32r)          # eye|ones|bkv|bq|eye16
            xt = sb.tile([128, NDT, S], bf16)          # x^T tiles
            kvt = sb.tile([128, S], bf16)              # rows 0:64 K^T, 64:128 V^T
            vT32 = sb.tile([64, S], f32r)              # V^T rows, f32 (for PE transpose)
            qt_sb = sb.tile([128, QW], bf16)           # rows 0:64 Q^T
            vnat = sb.tile([128, NKT, 65], bf16)       # V natural + ones col
            yT_sb = sb.tile([128, QW], f32)
            warm_sb = sb.tile([128, 4], f32)
            junk_sb = sb.tile([128, 512], bf16)   # never written: junk warms

            bkv_sb = cst_sb[:, 130:131].bitcast(f32)
            bq_sb = cst_sb[:, 131:132].bitcast(f32)
            id_sb = cst_sb[:, 0:128]                   # f32r eye (transposes)
            id16 = cst_sb[:, 132:196].bitcast(bf16)    # [128, 128] bf16 eye
            w_fl = w_sb.rearrange("p t m -> p (t m)")

            for _rep in range(repeats):
              with tc.tile_pool(name=f"pa{_rep}", bufs=1, space="PSUM") as pa:
                # one SP-queue stream, arrival order = need order:
                # chunk 0 first (longest transfer on the critical path),
                # then w + consts, then chunks 1..7
                xT_r = xT_d.rearrange("(t p) s -> p t s", p=128)
                nc.sync.dma_start(xt[:, :, 0:512], xT_r[:, :, 0:512])
                nc.sync.dma_start(w_sb, w_d.rearrange("(t p) m -> p t m",
                                                      p=128))
                nc.sync.dma_start(cst_sb, cst_d)
                for c in range(1, NSC):
                    cs = slice(512 * c, 512 * (c + 1))
                    nc.sync.dma_start(xt[:, :, cs], xT_r[:, :, cs])

                # seed the junk/warm tiles via the idle Pool engine (tile
                # framework wants writers; Pool has no startup dependency,
                # so the PE junk warms can begin at t~0)
                nc.gpsimd.memset(junk_sb, 1.0)
                nc.gpsimd.memset(warm_sb, 1.0)
                # warm-ups: pre-touch operands one semaphore at a time (walrus
                # allows at most ONE sync wait per engine instruction)
                nc.scalar.activation(warm_sb[0:1, 2:3], warm_sb[0:1, 3:4], EXP,
                                     scale=1.0)
                nc.vector.tensor_copy(warm_sb[:, 0:1], bkv_sb)
                nc.vector.memset(vnat[:, :, 64:65], 1.0)
                warm = pa.tile([128, HW], f32, tag="st", bufs=2)
                # HAM warm-up: junk matmuls on an UNWRITTEN sbuf tile (no DMA
                # wait -> PE busy from t~0) keep the p-state ramp running
                # through the w/x^T chunk-0 DMA waits so the head projections
                # run as fast as possible
                for _ in range(N_JUNK):
                    nc.tensor.matmul(warm[:, 0:512], lhsT=junk_sb[:, 0:128],
                                     rhs=junk_sb, start=True, stop=True)

                def proj_kv(c, tag="st"):
                    cs = slice(512 * c, 512 * (c + 1))
                    pkv = pa.tile([128, HW], f32, tag=tag,
                                  bufs=(2 if tag == "st" else 1), name="pkv")
                    for dt in range(NDT):
                        nc.tensor.matmul(
                            pkv[:, 0:512],
                            lhsT=w_sb[:, dt, 0:128], rhs=xt[:, dt, cs],
                            start=(dt == 0), stop=(dt == NDT - 1))
                    nc.vector.tensor_scalar_add(kvt[:, cs], pkv[:, 0:512],
                                                bkv_sb)
                    nc.vector.tensor_scalar_add(
                        vT32[:, cs], pkv[64:128, 0:512],
                        bkv_sb[64:128, :])

                def proj_q(c, tag="st"):
                    cs = slice(512 * c, 512 * (c + 1))
                    pq = pa.tile([128, HW], f32, tag=tag,
                                 bufs=(2 if tag == "st" else 1), name="pq")
                    for dt in range(NDT):
                        nc.tensor.matmul(
                            pq[0:D_H, 0:512],
                            lhsT=w_sb[:, dt, 128:192], rhs=xt[:, dt, cs],
                            start=(dt == 0), stop=(dt == NDT - 1))
                    nc.vector.tensor_scalar_add(
                        qt_sb[0:D_H, cs], pq[0:D_H, 0:512], bq_sb[0:D_H, :])

                def v_nat(c):
                    pvt = pa.tile([128, HW], f32r, tag="outB", bufs=1,
                                  name="pvt")
                    for j in range(4):
                        kt = 4 * c + j
                        nc.tensor.transpose(
                            pvt[:, D_H * j:D_H * (j + 1)],
                            in_=vT32[:, 128 * kt:128 * (kt + 1)],
                            identity=id_sb[0:64, 0:64])
                    nc.vector.tensor_copy(
                        vnat[:, 4 * c:4 * (c + 1), 0:D_H],
                        pvt[:, 0:4 * D_H].rearrange("p (t h) -> p t h", h=D_H))
                    # junk matmul: advances the PE engine clock past the vnat
                    # copy's DVE tick (walrus 1-wait limit on later AV MMs)
                    nc.tensor.matmul(
                        pvt[0:65, 512:514].bitcast(f32), lhsT=vnat[:, 4 * c, :],
                        rhs=vnat[:, 4 * c, 0:2], start=True, stop=True)

                def chunk_work(c):
                    # kv projection + V transpose of one s-chunk in a single
                    # outB slot hold
                    cs = slice(512 * c, 512 * (c + 1))
                    t = pa.tile([128, HW], f32, tag="outB", bufs=1, name="cw")
                    for dt in range(NDT):
                        nc.tensor.matmul(
                            t[:, 0:512],
                            lhsT=w_sb[:, dt, 0:128], rhs=xt[:, dt, cs],
                            start=(dt == 0), stop=(dt == NDT - 1))
                    nc.vector.tensor_scalar_add(kvt[:, cs], t[:, 0:512],
                                                bkv_sb)
                    nc.vector.tensor_scalar_add(
                        vT32[:, cs], t[64:128, 0:512],
                        bkv_sb[64:128, :])
                    for j in range(4):
                        kt = 4 * c + j
                        nc.tensor.transpose(
                            t[:, 512 + D_H * j:512 + D_H * (j + 1)]
                            .bitcast(f32r),
                            in_=vT32[:, 128 * kt:128 * (kt + 1)],
                            identity=id_sb[0:64, 0:64])
                    nc.vector.tensor_copy(
                        vnat[:, 4 * c:4 * (c + 1), 0:D_H],
                        t[:, 512:512 + 4 * D_H].bitcast(f32r)
                        .rearrange("p (t h) -> p t h", h=D_H))
                    nc.tensor.matmul(
                        t[0:65, 768:770], lhsT=vnat[:, 4 * c, :],
                        rhs=vnat[:, 4 * c, 0:2], start=True, stop=True)

                # minimal head: only what S^T(kt0, h0) needs, so the ACT
                # exp stream starts as early as the x^T q-column DMAs allow
                proj_kv(0)
                proj_q(0)
                proj_q(1)

                poutA = pa.tile([65, HW], f32, tag="out", name="poutA")

                def st_tile(kt, h, tag="st"):
                    pst = pa.tile([128, HW], f32, tag=tag,
                                  bufs=(2 if tag == "st" else 1),
                                  name=f"pst_{kt}_{h}")
                    for c in range(2):
                        cs = slice(512 * c, 512 * (c + 1))
                        qs = slice(HW * h + 512 * c, HW * h + 512 * (c + 1))
                        nc.tensor.matmul(
                            pst[:, cs],
                            lhsT=kvt[0:64, 128 * kt:128 * (kt + 1)],
                            rhs=qt_sb[0:64, qs],
                            start=True, stop=True)
                    return pst

                # flat job order: all h=0 (kt 0..31), then all h=1
                jobs = [(k, 0) for k in range(NKT)] + [(k, 1) for k in
                                                       range(NKT)]
                NJ = 2 * NKT
                last_h0 = NKT - 1  # job 31

                # deadline-paced extras inside the h=0 jobs.  chunk c's K^T
                # is needed when S^T(4c, h0) is EMITTED (job 4c-2), its V by
                # AV(4c) (job 4c); q chunks 2-3 by the first h=1 S^T emission
                # (job 30); all outB users release before poutB allocates
                # (job 32).
                extra_at = {
                    0: lambda: v_nat(0),
                    1: lambda: proj_kv(1, tag="outB"),
                    2: lambda: v_nat(1),
                    5: lambda: chunk_work(2),
                    9: lambda: chunk_work(3),
                    13: lambda: chunk_work(4),
                    17: lambda: chunk_work(5),
                    21: lambda: chunk_work(6),
                    23: lambda: proj_q(2, tag="outB"),
                    25: lambda: chunk_work(7),
                    27: lambda: proj_q(3, tag="outB"),
                }

                def ship(h, pieces, engines=("v",)):
                    # pouts[h] -> SBUF -> DRAM, split for copy/DMA overlap;
                    # engines rotates the copy over DVE ("v") / ACT ("s")
                    w_ = HW // pieces
                    for p in range(pieces):
                        ps = slice(HW * h + w_ * p, HW * h + w_ * (p + 1))
                        ls = slice(w_ * p, w_ * (p + 1))
                        if engines[p % len(engines)] == "v":
                            nc.vector.tensor_copy(yT_sb[0:65, ps],
                                                  pouts[h][:, ls])
                        else:
                            nc.scalar.copy(yT_sb[0:65, ps], pouts[h][:, ls])
                        nc.sync.dma_start(yT_d[:, ps], yT_sb[0:65, ps])

                pouts = [poutA, None]
                psts = {0: st_tile(*jobs[0]), 1: st_tile(*jobs[1])}
                # warm the matmul-weight-load and transpose paths off the
                # critical chain (pre-touch w/id16 one semaphore at a time)
                nc.tensor.matmul(warm[:, 0:2], lhsT=w_sb[:, 0, 0:128],
                                 rhs=w_sb[:, 0, 0:2], start=True, stop=True)
                nc.tensor.transpose(warm[0:1, 4:132].bitcast(f32r),
                                    in_=id_sb[:, 0:1], identity=id_sb)
                def av(kt, h, pt):
                    for cc in range(2):
                        cs = slice(512 * cc, 512 * (cc + 1))
                        nc.tensor.matmul(
                            pouts[h][:, cs],
                            lhsT=vnat[:, kt, :],
                            rhs=pt[:, cs],
                            start=(kt == 0), stop=(kt == NKT - 1),
                            skip_group_check=True)

                deferred = []   # (j_emitted, kt, h, pt) awaiting their AV
                for j in range(NJ):
                    kt, h = jobs[j]
                    if h == 1 and pouts[1] is None:
                        pouts[1] = pa.tile([65, HW], f32, tag="out",
                                           name="poutB")
                    pt = ptp.tile([128, HW], bf16, tag="pt", name="ptile")
                    if j in OFFLOAD_EXP:
                        # exp on DVE: 2-pass (1+z/1024)^1024; AV deferred two
                        # jobs so the PE stream never waits on DVE latency
                        mid = ptp.tile([128, HW], f32, tag="mid", bufs=2,
                                       name="mid")
                        nc.vector._custom_dve(EXPOP, out=mid, in0=psts.pop(j),
                                              s0=EXPC0, s1=1.0)
                        nc.vector._custom_dve(EXPOP, out=pt, in0=mid,
                                              s0=1.0, s1=0.0)
                    else:
                        nc.scalar.activation(pt, psts.pop(j), EXP, scale=0.125)
                    if j == 0:
                        psts[2] = st_tile(*jobs[2])
                    if j in extra_at:
                        extra_at.pop(j)()
                    while deferred and deferred[0][0] <= j - 2:
                        _, dk, dh, dpt = deferred.pop(0)
                        av(dk, dh, dpt)
                    if j in OFFLOAD_EXP:
                        deferred.append((j, kt, h, pt))
                    else:
                        av(kt, h, pt)
                    if j == last_h0:
                        ship(0, pieces=1)
                    if j + 3 < NJ:
                        m = j + 3
                        tag = "outB" if (m >= 33 and m % 3 == 0) else "st"
                        psts[m] = st_tile(*jobs[m], tag=tag)
                assert not extra_at and not deferred
                ship(1, pieces=2, engines=("s", "v"))

    nc.compile()
    return nc


def _prep_core_inputs(c, x, Wq, bq, Wk, bk, Wv, bv):
    import ml_dtypes
    bf16 = ml_dtypes.bfloat16
    b, qh = c // 2, c % 2
    xb = x[b]
    if qh:
        xb = np.concatenate([xb[QW:], xb[:QW]], axis=0)
    consts = np.zeros((128, CW), np.float32)
    consts[:, 0:128] = np.eye(128, dtype=np.float32)
    consts[:, 128] = 1.0
    # col 129: two packed bf16 ones (vnat denominator column source)
    consts[:, 130] = np.concatenate([bk, bv])  # [bk;bv] per-partition bias
    consts[0:D_H, 131] = bq
    # cols 132:196: bf16 eye(128), packed 2-per-f32
    eye16 = np.eye(128, dtype=bf16).view(np.uint16).reshape(128, 64, 2)
    packed = (eye16[:, :, 0].astype(np.uint32)
              | (eye16[:, :, 1].astype(np.uint32) << 16))
    consts[:, 132:196] = packed.view(np.float32)
    # cols 196:212: 32 packed bf16 ones
    consts[:, 196:212] = np.array([0x3F803F80], np.uint32).view(np.float32)[0]
    return {
        "xT": np.ascontiguousarray(xb.T).astype(bf16),
        "w": np.ascontiguousarray(
            np.concatenate([Wk, Wv, Wq], axis=1)).astype(bf16),
        "consts": consts,
    }


def gather_output(per_core_yT):
    """per_core_yT: list of 8 arrays [65, QW] -> full y [B, S, D_H]."""
    y = np.empty((B, S, D_H), np.float32)
    for c in range(N_CORES):
        b, qh = c // 2, c % 2
        yT = np.asarray(per_core_yT[c])
        y[b, qh * QW:(qh + 1) * QW] = (yT[0:D_H] / yT[D_H:D_H + 1]).T
    return y


def run(x, Wq, bq, Wk, bk, Wv, bv, trace=False):
    """Returns (y [B,S,H], BassKernelResults)."""
    from concourse import bass_utils

    x = np.asarray(x, np.float32)
    in_maps = [
        _prep_core_inputs(c, x, np.asarray(Wq, np.float32),
                          np.asarray(bq, np.float32), np.asarray(Wk, np.float32),
                          np.asarray(bk, np.float32), np.asarray(Wv, np.float32),
                          np.asarray(bv, np.float32))
        for c in range(N_CORES)
    ]
    nc = build_nc()
    res = bass_utils.run_bass_kernel_spmd(
        nc, in_maps, core_ids=list(range(N_CORES)), trace=trace)
    y = gather_output([res.results[c]["yT"] for c in range(N_CORES)])
    return y, res


def kernel(x, Wq, bq, Wk, bk, Wv, bv):
    y, _ = run(x, Wq, bq, Wk, bk, Wv, bv, trace=False)
    return y
